# revision 2
# baseline (speedup 1.0000x reference)
"""CoordGNN kernel for 8 Trainium2 NeuronCores.

Sharding strategy (per spec sharding_hint): destination nodes are
partitioned into 8 contiguous ranges, one per core. Host-side prep sorts
each graph's edges by dst and packs them into a dense dst-major padded
layout [n_dst, Dmax] so edge-softmax and scatter-sum become dense masked
reductions on-device (no segment primitives). Each core computes its dst
shard; layer-1 outputs are re-gathered on host and broadcast for the
layer-2 src-feature gathers (the halo is effectively the full table).
MLP weights are replicated to all cores.
"""
import numpy as np
import jax
import jax.numpy as jnp

NC = 8
IN_F, H, COORD = 16, 128, 2
N0, N1, N2 = 100000, 50000, 25000
E0, E1 = 1600000, 800000


# ----------------------------------------------------------------- host prep
def _prep_graph(src, dst, offsets, n_dst, nc=NC):
    """Pack edges into per-core dense dst-major padded arrays."""
    src = np.asarray(src).astype(np.int32)
    dst = np.asarray(dst).astype(np.int64)
    off = np.asarray(offsets, dtype=np.float32)
    order = np.argsort(dst, kind="stable")
    src_s, dst_s, off_s = src[order], dst[order], off[order]
    counts = np.bincount(dst_s, minlength=n_dst)
    dmax = int(counts.max())
    starts = np.zeros(n_dst + 1, np.int64)
    np.cumsum(counts, out=starts[1:])
    pos = np.arange(len(dst_s), dtype=np.int64) - starts[dst_s]

    src_p = np.zeros((n_dst, dmax), np.int32)
    off_p = np.zeros((n_dst, dmax, 2), np.float32)
    mask = np.zeros((n_dst, dmax), np.float32)
    src_p[dst_s, pos] = src_s
    off_p[dst_s, pos] = off_s
    mask[dst_s, pos] = 1.0

    ndp = n_dst // nc
    return (
        src_p.reshape(nc, ndp, dmax),
        off_p.reshape(nc, ndp, dmax, 2),
        mask.reshape(nc, ndp, dmax),
    )


# --------------------------------------------------------------- device math
def _lin(p, x):
    return x @ p["w"] + p["b"]


def _seq(ps, x):
    for i, p in enumerate(ps):
        x = _lin(p, x)
        if i < len(ps) - 1:
            x = jax.nn.elu(x)
    return x


def _conv_agg(p, table, off_p, src_p, mask):
    """Edge phase of one coord-conv for this core's dst shard.

    table   [n_src, F]    full (replicated) source-feature table
    off_p   [nd, D, 2]    padded per-slot coordinate offsets
    src_p   [nd, D]       padded per-slot src indices (0 for pads)
    mask    [nd, D]       1.0 for real edges
    returns [nd, F]       softmax-weighted scatter-sum aggregate
    """
    nd, dmax, _ = off_p.shape
    kw = _seq(p["kernel"], off_p.reshape(nd * dmax, COORD))
    kw = kw.reshape(nd, dmax, -1)

    score = 1.0 / (jnp.abs(off_p).sum(-1) + 0.001)
    score = jnp.where(mask > 0, score, -jnp.inf)
    smax = score.max(axis=1)
    smax = jnp.where(jnp.isfinite(smax), smax, 0.0)
    ex = jnp.exp(score - smax[:, None])
    denom = ex.sum(axis=1)
    w = ex / jnp.maximum(denom, 1e-20)[:, None]

    fs = table[src_p.reshape(-1)].reshape(nd, dmax, -1)
    m = fs * (w[:, :, None] * kw)
    return m.sum(axis=1)


def _phase1(params, feat_full, f0_c, off_c, src_c, mask_c):
    """Layer-1: three parallel coord-convs + linear skips for this shard."""
    outs = []
    for conv, skip in (("conv1", "skip1"), ("conv2", "skip2"), ("conv3", "skip3")):
        p = params[conv]
        agg = _conv_agg(p, feat_full, off_c, src_c, mask_c)
        hs = _seq(p["mlp_self"], f0_c)
        h = _seq(p["mlp"], jnp.concatenate([agg, hs], axis=1))
        outs.append(h + _lin(params[skip], f0_c))
    return tuple(outs)  # pre-ELU layer-1 features, 3 chains


def _phase2(params, h_fulls, h1_cs, f1_c, off_c, src_c, mask_c):
    """Layer-2 convs + output MLPs for this core's dst1 shard."""
    chains = (
        ("conv4", "skip4", "out1"),
        ("conv5", "skip5", "out2"),
        ("conv6", "skip6", "out3"),
    )
    outs = []
    for i, (conv, skip, outn) in enumerate(chains):
        p = params[conv]
        table = jax.nn.elu(h_fulls[i])  # layer-2 input features
        agg = _conv_agg(p, table, off_c, src_c, mask_c)
        nd = off_c.shape[0]
        # mlp_self input is elu(h)[this core's dst1 rows]
        row0 = jax.lax.axis_index("c") * nd
        self_in = jax.lax.dynamic_slice_in_dim(table, row0, nd, 0)
        hs = _seq(p["mlp_self"], self_in)
        h2 = _seq(p["mlp"], jnp.concatenate([agg, hs], axis=1)) + h1_cs[i]
        hcat = jax.nn.elu(
            jnp.concatenate([h2, _lin(params[skip], f1_c)], axis=1)
        )
        outs.append(_seq(params[outn], hcat))
    return jnp.concatenate(outs, axis=1)  # [nd, 4]


# -------------------------------------------------------------------- kernel
def kernel(feat, offsets0, offsets1, src0, dst0, src1, dst1, n_dst0, n_dst1, params):
    feat = np.asarray(feat, np.float32)
    params = jax.tree_util.tree_map(lambda a: np.asarray(a, np.float32), params)
    n1, n2 = int(n_dst0), int(n_dst1)

    src0_p, off0_p, mask0 = _prep_graph(src0, dst0, offsets0, n1)
    src1_p, off1_p, mask1 = _prep_graph(src1, dst1, offsets1, n2)

    ndp1, ndp2 = n1 // NC, n2 // NC
    f0 = feat[:n1].reshape(NC, ndp1, IN_F)
    f1 = feat[:n2].reshape(NC, ndp2, IN_F)
    feat_rep = np.broadcast_to(feat, (NC,) + feat.shape)

    p1 = jax.pmap(_phase1, axis_name="c", in_axes=(None, 0, 0, 0, 0, 0))
    h_a, h_b, h_c = p1(params, feat_rep, f0, off0_p, src0_p, mask0)
    h_fulls = np.stack(
        [np.asarray(h_a).reshape(n1, H),
         np.asarray(h_b).reshape(n1, H),
         np.asarray(h_c).reshape(n1, H)], axis=0
    )  # [3, n1, H] pre-ELU

    h1_cs = h_fulls[:, :n2].reshape(3, NC, ndp2, H).transpose(1, 0, 2, 3)
    h_fulls_rep = np.broadcast_to(h_fulls, (NC,) + h_fulls.shape)

    p2 = jax.pmap(_phase2, axis_name="c", in_axes=(None, 0, 0, 0, 0, 0, 0))
    out = p2(params, h_fulls_rep, h1_cs, f1, off1_p, src1_p, mask1)
    return np.asarray(out).reshape(n2, 4).astype(np.float32)


# revision 3
# speedup vs baseline: 6.2083x; 6.2083x over previous
"""CoordGNN kernel for 8 Trainium2 NeuronCores.

Sharding strategy (per spec sharding_hint): destination nodes are
partitioned into 8 contiguous ranges, one per core. Host-side prep sorts
each graph's edges by dst and packs them into a dense dst-major padded
layout [n_dst, Dmax] so edge-softmax and scatter-sum become dense masked
reductions on-device (no segment primitives). The whole network runs in
a single pmap: layer-1 shards are exchanged with lax.all_gather (the
layer-2 src halo is effectively the full table). MLP weights replicated.
"""
import numpy as np
import jax
import jax.numpy as jnp

NC = 8
IN_F, H, COORD = 16, 128, 2
N0, N1, N2 = 100000, 50000, 25000


# ----------------------------------------------------------------- host prep
def _prep_graph(src, dst, offsets, n_dst, nc=NC):
    """Pack edges into per-core dense dst-major padded arrays."""
    src = np.asarray(src).astype(np.int32)
    dst = np.asarray(dst).astype(np.int64)
    off = np.asarray(offsets, dtype=np.float32)
    order = np.argsort(dst, kind="stable")
    src_s, dst_s, off_s = src[order], dst[order], off[order]
    counts = np.bincount(dst_s, minlength=n_dst)
    dmax = int(counts.max())
    starts = np.zeros(n_dst + 1, np.int64)
    np.cumsum(counts, out=starts[1:])
    pos = np.arange(len(dst_s), dtype=np.int64) - starts[dst_s]

    src_p = np.zeros((n_dst, dmax), np.int32)
    off_p = np.zeros((n_dst, dmax, 2), np.float32)
    mask = np.zeros((n_dst, dmax), np.float32)
    src_p[dst_s, pos] = src_s
    off_p[dst_s, pos] = off_s
    mask[dst_s, pos] = 1.0

    ndp = n_dst // nc
    return (
        src_p.reshape(nc, ndp, dmax),
        off_p.reshape(nc, ndp, dmax, 2),
        mask.reshape(nc, ndp, dmax),
    )


# --------------------------------------------------------------- device math
def _lin(p, x):
    return x @ p["w"] + p["b"]


def _seq(ps, x):
    for i, p in enumerate(ps):
        x = _lin(p, x)
        if i < len(ps) - 1:
            x = jax.nn.elu(x)
    return x


def _conv_agg(p, table, off_p, src_p, mask):
    """Edge phase of one coord-conv for this core's dst shard."""
    nd, dmax, _ = off_p.shape
    kw = _seq(p["kernel"], off_p.reshape(nd * dmax, COORD))
    kw = kw.reshape(nd, dmax, -1)

    score = 1.0 / (jnp.abs(off_p).sum(-1) + 0.001)
    score = jnp.where(mask > 0, score, -jnp.inf)
    smax = score.max(axis=1)
    smax = jnp.where(jnp.isfinite(smax), smax, 0.0)
    ex = jnp.exp(score - smax[:, None])
    denom = ex.sum(axis=1)
    w = ex / jnp.maximum(denom, 1e-20)[:, None]

    fs = table[src_p.reshape(-1)].reshape(nd, dmax, -1)
    m = fs * (w[:, :, None] * kw)
    return m.sum(axis=1)


def _network(params, feat_shard, off0, src0, mask0, off1, src1, mask1):
    c = jax.lax.axis_index("c")
    feat = jax.lax.all_gather(feat_shard, "c").reshape(N0, IN_F)
    nd1, nd2 = N1 // NC, N2 // NC
    f0 = jax.lax.dynamic_slice_in_dim(feat, c * nd1, nd1, 0)
    f1 = jax.lax.dynamic_slice_in_dim(feat, c * nd2, nd2, 0)

    # ---- layer 1: three parallel coord-convs + linear skips (this shard)
    hs1 = []
    for conv, skip in (("conv1", "skip1"), ("conv2", "skip2"), ("conv3", "skip3")):
        p = params[conv]
        agg = _conv_agg(p, feat, off0, src0, mask0)
        selfh = _seq(p["mlp_self"], f0)
        h = _seq(p["mlp"], jnp.concatenate([agg, selfh], axis=1))
        hs1.append(h + _lin(params[skip], f0))

    # ---- halo exchange + layer 2 + output MLPs
    outs = []
    chains = (
        ("conv4", "skip4", "out1"),
        ("conv5", "skip5", "out2"),
        ("conv6", "skip6", "out3"),
    )
    for i, (conv, skip, outn) in enumerate(chains):
        p = params[conv]
        h_full = jax.lax.all_gather(hs1[i], "c").reshape(N1, H)  # pre-ELU
        h1_c = jax.lax.dynamic_slice_in_dim(h_full, c * nd2, nd2, 0)
        table = jax.nn.elu(h_full)
        agg = _conv_agg(p, table, off1, src1, mask1)
        self_in = jax.lax.dynamic_slice_in_dim(table, c * nd2, nd2, 0)
        selfh = _seq(p["mlp_self"], self_in)
        h2 = _seq(p["mlp"], jnp.concatenate([agg, selfh], axis=1)) + h1_c
        hcat = jax.nn.elu(jnp.concatenate([h2, _lin(params[skip], f1)], axis=1))
        outs.append(_seq(params[outn], hcat))
    return jnp.concatenate(outs, axis=1)  # [nd2, 4]


_pmapped = None


def _get_pmapped():
    global _pmapped
    if _pmapped is None:
        _pmapped = jax.pmap(
            _network, axis_name="c", in_axes=(None, 0, 0, 0, 0, 0, 0, 0)
        )
    return _pmapped


# -------------------------------------------------------------------- kernel
def kernel(feat, offsets0, offsets1, src0, dst0, src1, dst1, n_dst0, n_dst1, params):
    feat = np.asarray(feat, np.float32)
    params = jax.tree_util.tree_map(lambda a: np.asarray(a, np.float32), params)
    n1, n2 = int(n_dst0), int(n_dst1)

    src0_p, off0_p, mask0 = _prep_graph(src0, dst0, offsets0, n1)
    src1_p, off1_p, mask1 = _prep_graph(src1, dst1, offsets1, n2)
    feat_sh = feat.reshape(NC, N0 // NC, IN_F)

    out = _get_pmapped()(
        params, feat_sh, off0_p, src0_p, mask0, off1_p, src1_p, mask1
    )
    return np.asarray(out).reshape(n2, 4).astype(np.float32)
